# revision 16
# baseline (speedup 1.0000x reference)
"""Trainium2 Bass kernel for ChannelDepsModule (sequential channel recurrence).

Math (per pixel, fp32):
    m_0 = mix_0 ; ybar_0 = round(x_0 - m_0) + m_0
    for i in 1..191:
        m_i = sum_{c<i} Wm[i-1,c] * ybar_c + b[i-1] + mix_i
        ybar_i = round(x_i - m_i) + m_i
    outputs: ybar, mix_out (= m)

Device strategy (per core, one batch image, 4096 pixels):
  - pixels on SBUF partitions ([128] x 32 chunks), channels on the free dim
  - channels in 6 blocks of 32:
      * cross-block mix contributions P via TensorE matmuls
        (stationary ybar in channel-partition layout x Wm^T slice), with
        pixel-partition PSUM output; q = x - mix - b - P is written into the
        block's ybar columns ahead of time
      * in-block recurrence: one fused DVE scan per channel computes
        t_i = q_i - sum_j w_ij y_j directly (weights negated, +1 planted on
        the diagonal so the prefilled q column enters the dot), then one
        fused DVE op assembles y_i = round(t_i) + (x_i - t_i) using the
        +-1.5*2^23 magic constant (IEEE RNE == jnp.round)
      * mix_out column m_i = x_i - t_i is produced on the GpSimd engine,
        off the critical path
      * finished ybar columns are TensorE-transposed (two half-block waves)
        into channel-partition tiles for later blocks' matmuls
  - b is folded into mix on the host; mix_out channel 0 restored on host
"""

import sys

import numpy as np

if "/opt/trn_rl_repo" not in sys.path:
    sys.path.insert(0, "/opt/trn_rl_repo")

N, C, H, Wd = 8, 192, 64, 64
NPIX = H * Wd          # 4096 pixels per core
B = 32                 # channel block size
NBLK = C // B          # 6
ROUND_C = 1.5 * 2.0**23  # fp32 add of this rounds to nearest-even integer

_CACHE = {}
_DVE_OPS = {}


def _register_dve_ops():
    """Define + register the two fused DVE ops (idempotent)."""
    if _DVE_OPS:
        return _DVE_OPS
    import concourse.dve_ops as dops
    import concourse.dve_spec as ds
    from concourse.dve_spec import AluOp, Spec, Src0, Src1
    from concourse.dve_ops import CUSTOM_DVE_SPECS, OPS, DveOp
    from concourse.dve_uop import DveOpSpec

    # The stock segmented-scan machinery only implements the page-counter
    # mode; add the documented per-page *reset* behavior for scans marked
    # with `_page_reset`: at each SUB_DIM_DONE the STEP state computes
    # d <- op(init, expr) instead of op(CURR, expr).
    if not getattr(ds, "_page_reset_patched", False):
        _orig = ds._scan_overrides

        def _patched(scans, node_stage):
            seed, step = _orig(scans, node_stage)
            for sc in scans:
                if getattr(sc, "_page_reset", False):
                    d = node_stage[sc]
                    step[d] = ds._Stage(sc.op, ds._scan_init(sc), sc.expr)
            return seed, step

        ds._scan_overrides = _patched
        ds._page_reset_patched = True

    def _chaindot_ref(in0, in1, s0, s1, imm2):
        p = in0.shape[0]
        inner = in0.shape[-1]
        a = in0.reshape(p, -1, inner).astype(np.float32)
        bb = in1.reshape(p, -1, inner).astype(np.float32)
        return np.cumsum(a * bb, axis=-1, dtype=np.float32).reshape(in0.shape)

    sc = ds.scan(AluOp.ADD, Src0 * Src1)
    object.__setattr__(sc, "_page_reset", True)
    spec_cd = Spec(body=sc, reference=_chaindot_ref)

    def _quanty_ref(in0, in1, s0, s1, imm2):
        c = np.float32(s0)
        t = in0.astype(np.float32)
        return ((t + c) - c) + (in1.astype(np.float32) - t)

    spec_qy = Spec(
        body=((Src0 + ds.C0) - ds.C0) + (Src1 - Src0), reference=_quanty_ref
    )

    def _mk(name, spec, subdim):
        if any(o.name == name for o in OPS):
            op = next(o for o in OPS if o.name == name)
        else:
            shas = {}
            for ver in ("v3", "v4"):
                shas[ver] = DveOpSpec(
                    name=name, uops=ds.lower(spec, ver=ver)
                ).sha(ver)
            op = DveOp(name, spec, subdim=subdim, uops_sha=shas)
            OPS.append(op)
            CUSTOM_DVE_SPECS[name] = spec
            dops._SUB_OPCODE_FOR_NAME[name] = dops._CUSTOM_DVE_ROW_BASE + len(OPS) - 1
        return op

    _DVE_OPS["chaindot"] = _mk("CHAINDOT_SEQ_ANT", spec_cd, subdim=True)
    _DVE_OPS["quanty"] = _mk("QUANTY_ANT", spec_qy, subdim=False)
    return _DVE_OPS


def _build(n_chunks):
    """Build + compile the per-core Bass module. n_chunks pixel chunks of 128."""
    import concourse.bacc as bacc
    import concourse.mybir as mybir
    from concourse.tile import TileContext

    ops = _register_dve_ops()
    npix = n_chunks * 128
    fp32 = mybir.dt.float32
    K = n_chunks  # pixel chunks

    nc = bacc.Bacc(None, target_bir_lowering=False)

    # dram layout: row (b, p) holds the [K, B] slab of block b, lane p —
    # 4KB contiguous runs on both sides of the DMA, and block-columns of
    # any 4 chunks are contiguous in SBUF (single-free-dim transpose APs)
    xt = nc.dram_tensor("xt", [NBLK * 128, K * B], fp32, kind="ExternalInput")
    mixt = nc.dram_tensor("mixt", [NBLK * 128, K * B], fp32, kind="ExternalInput")
    wt = nc.dram_tensor("wt", [C, C], fp32, kind="ExternalInput")
    wtri = nc.dram_tensor("wtri", [1, NBLK * B * B], fp32, kind="ExternalInput")
    ident = nc.dram_tensor("ident", [128, 128], fp32, kind="ExternalInput")
    yt = nc.dram_tensor("yt", [NBLK * 128, K * B], fp32, kind="ExternalOutput")
    mot = nc.dram_tensor("mot", [NBLK * 128, K * B], fp32, kind="ExternalOutput")

    with TileContext(nc) as tc:
        with (
            tc.tile_pool(name="big", bufs=1) as big,
            tc.tile_pool(name="small", bufs=1) as small,
            tc.tile_pool(name="scr", bufs=3) as scr,
            tc.tile_pool(name="qp", bufs=2) as qp,
            tc.tile_pool(name="psum_e", bufs=2, space="PSUM") as psum_e,
            tc.tile_pool(name="psum_d", bufs=1, space="PSUM") as psum_d,
            tc.tile_pool(name="psumt", bufs=2, space="PSUM") as psumt,
        ):
            # pixel-partition tiles, free layout = k*192 + c
            X = big.tile([128, K * C], fp32, tag="X")
            MIX = big.tile([128, K * C], fp32, tag="MIX")  # becomes mix_out
            XMB = big.tile([128, K * C], fp32, tag="XMB")
            Y = big.tile([128, K * C], fp32, tag="Y")
            # channel-partition decoded ybar: chans 0-127 / 128-159
            ysb_lo = big.tile([128, npix], fp32, tag="ysb_lo")
            ysb_hi = big.tile([32, npix], fp32, tag="ysb_hi")

            wt_lo = small.tile([128, C], fp32, tag="wt_lo")
            wt_hi = small.tile([64, C], fp32, tag="wt_hi")
            wtri_t = small.tile([1, NBLK * B * B], fp32, tag="wtri")
            wtri_b = small.tile([128, NBLK * B * B], fp32, tag="wtri_b")
            id_t = small.tile([128, 128], fp32, tag="ident")

            def big_in(tile, dram):
                nc.sync.dma_start(
                    tile[:].rearrange("p (b f) -> p b f", f=K * B),
                    dram[:].rearrange("(b p) f -> p b f", p=128),
                )

            big_in(X, xt)
            big_in(MIX, mixt)
            nc.sync.dma_start(wt_lo[:], wt[0:128, :])
            nc.sync.dma_start(wt_hi[:], wt[128:C, :])
            nc.sync.dma_start(wtri_t[:], wtri[:])
            nc.sync.dma_start(id_t[:], ident[:])
            nc.gpsimd.partition_broadcast(wtri_b[:], wtri_t[:])

            # XMB = X - (MIX + b)  (b folded into MIX on host)
            nc.vector.tensor_sub(XMB[:], X[:], MIX[:])

            def col(tile, ch):  # strided [128, K] view of channel ch
                b, c = divmod(ch, B)
                return tile[:].rearrange("p (b k c) -> p b k c", c=B, k=K)[
                    :, b, :, c
                ]

            def ycols(sb, j0, j1):  # [128, K, j1-j0] view of block sb's cols
                return Y[:].rearrange("p (b k c) -> p b k c", c=B, k=K)[
                    :, sb, :, j0:j1
                ]

            def xmb_slice(sb):
                return XMB[:].rearrange("p (b k c) -> p b k c", c=B, k=K)[
                    :, sb, :, :
                ]

            # PSUM accumulation-group bookkeeping: start=True marks the whole
            # 2KB bank pending-zero, so the first matmul touching each bank
            # opens the group (later writes to untouched bytes overwrite,
            # rewrites accumulate) and the last one per bank closes it.
            BANK_CHUNKS = 512 // B  # chunk-slices per PSUM bank

            def p_early(sb, pp):
                """Early P for block sb: chans [0, 32(sb-1)) — one matmul/chunk."""
                kdec = (sb - 1) * B
                for k in range(K):
                    nc.tensor.matmul(
                        pp[:, k * B : (k + 1) * B],
                        ysb_lo[0:kdec, k * 128 : (k + 1) * 128],
                        wt_lo[0:kdec, sb * B : (sb + 1) * B],
                        start=(k % BANK_CHUNKS == 0),
                        stop=False,
                    )

            def p_final(sb, pp, had_early):
                """Final P seg for block sb: block sb-1's 32 chans."""
                r0 = (sb - 1) * B
                ys, wtile, rr = (
                    (ysb_lo, wt_lo, r0) if r0 < 128 else (ysb_hi, wt_hi, r0 - 128)
                )
                tp = (rr, 0) if rr not in (0, 32, 64) else None
                for k in range(K):
                    nc.tensor.matmul(
                        pp[:, k * B : (k + 1) * B],
                        ys[rr : rr + B, k * 128 : (k + 1) * 128],
                        wtile[rr : rr + B, sb * B : (sb + 1) * B],
                        start=(not had_early) and (k % BANK_CHUNKS == 0),
                        stop=(k % BANK_CHUNKS == BANK_CHUNKS - 1) or (k == K - 1),
                        tile_position=tp,
                    )

            def q_fin(sb, pp):
                """Block sb's Y cols = XMB - PP."""
                nc.vector.tensor_sub(
                    ycols(sb, 0, B),
                    xmb_slice(sb),
                    pp[:].rearrange("p (k c) -> p k c", c=B),
                )

            def transpose_block(sb):
                """Transpose Y cols of block sb into ysb (chan-part).

                4 pixel-chunks per TensorE transpose ([128,128] stationary,
                one identity stream instead of four); the chunk-interleaved
                PSUM result is de-interleaved by four [32,128] ScalarE
                copies into the same ysb layout as before.
                """
                base = sb * B
                if base < 128:
                    dst, dr0 = ysb_lo, base
                else:
                    dst, dr0 = ysb_hi, base - 128
                slab = Y[:].rearrange("p (b f) -> p b f", f=K * B)[:, sb, :]
                for g in range(0, K, 4):
                    pt = psumt.tile([128, 128], fp32, tag="pt")
                    nc.tensor.transpose(
                        pt[:],
                        slab[:, g * B : (g + 4) * B],
                        id_t[:],
                    )
                    for j in range(4):
                        nc.scalar.copy(
                            dst[dr0 : dr0 + B, (g + j) * 128 : (g + j + 1) * 128],
                            pt[j * B : (j + 1) * B, :],
                        )

            # tiny matmul reading the just-produced Y column: keeps the PE
            # HAM activity monitor from seeing a fully-idle MID window
            # during the long DVE scans (idle window -> K=4/8 = 1.2 GHz PE
            # clock for the *next* matmul burst). Issued in program order
            # after the block's p_early so it never head-of-line blocks it.
            def keepalive(ch):
                dpt = psum_d.tile([1, 8], fp32, tag="dummy")
                nc.tensor.matmul(
                    dpt[0:1, 0:1], col(Y, ch)[:, 0:1], wt_lo[:, 0:1],
                    start=True, stop=True,
                )

            def steps(sb):
                base = sb * B
                for i in range(B):
                    ch = base + i
                    if i > 0:
                        prod = scr.tile([128, B * K], fp32, tag="prod")
                        pr = prod[:].rearrange("p (k c) -> p k c", c=B)[
                            :, :, 0 : i + 1
                        ]
                        woff = sb * B * B + i * B
                        wrow = (
                            wtri_b[:, woff : woff + i + 1]
                            .unsqueeze(1)
                            .broadcast_to([128, K, i + 1])
                        )
                        nc.vector._custom_dve(
                            ops["chaindot"], out=pr, in0=ycols(sb, 0, i + 1),
                            in1=wrow,
                        )
                        t_ap = prod[:].rearrange("p (k c) -> p k c", c=B)[:, :, i]
                    else:
                        t_ap = col(Y, ch)
                    # mix_out column (off critical path, on GpSimd)
                    nc.gpsimd.tensor_sub(col(MIX, ch), col(X, ch), t_ap)
                    # y = round(t) + (x - t)
                    nc.vector._custom_dve(
                        ops["quanty"], out=col(Y, ch), in0=t_ap,
                        in1=col(X, ch), s0=ROUND_C,
                    )
                    if i % 4 == 2:
                        keepalive(ch)

            # ---------------- schedule ----------------
            pp_cur = None
            for sb in range(NBLK):
                if sb == 0:
                    nc.vector.tensor_copy(ycols(sb, 0, B), xmb_slice(sb))
                else:
                    q_fin(sb, pp_cur)
                if sb + 1 < NBLK:
                    pp = psum_e.tile([128, B * K], fp32, tag="pp")
                    if sb >= 1:  # overlaps this block's steps
                        p_early(sb + 1, pp)
                else:
                    pp = None
                steps(sb)
                if sb + 1 < NBLK:
                    transpose_block(sb)
                    p_final(sb + 1, pp, had_early=sb >= 1)
                pp_cur = pp

            def big_out(dram, tile):
                nc.sync.dma_start(
                    dram[:].rearrange("(b p) f -> p b f", p=128),
                    tile[:].rearrange("p (b f) -> p b f", f=K * B),
                )

            big_out(yt, Y)
            big_out(mot, MIX)

    nc.compile()
    return nc


def get_nc(n_chunks=NPIX // 128):
    if n_chunks not in _CACHE:
        _CACHE[n_chunks] = _build(n_chunks)
    return _CACHE[n_chunks]


def make_core_inputs(x, mix, W, b):
    """Host-side layout prep. Returns list of per-core input dicts."""
    Wm = (W * np.tril(np.ones((C - 1, C), np.float32))).astype(np.float32)
    wt = np.zeros((C, C), np.float32)
    wt[:, 1:] = Wm.T  # wt[c, i] = Wm[i-1, c]
    # in-block triangle, negated, with +1 on the diagonal: the scan over
    # [y_0..y_{i-1}, q_i] then yields t_i = q_i - sum_j w_ij y_j directly
    wtri = np.zeros((NBLK, B, B), np.float32)
    for sb in range(NBLK):
        for i in range(1, B):
            ch = sb * B + i
            wtri[sb, i, :i] = -Wm[ch - 1, sb * B : sb * B + i]
            wtri[sb, i, i] = 1.0
    wtri = wtri.reshape(1, -1)
    bpad = np.zeros((C,), np.float32)
    bpad[1:] = b
    ident = np.eye(128, dtype=np.float32)

    def to_slab(a):  # [C, H, W] -> [(b p), (k c)]
        return np.ascontiguousarray(
            a.reshape(NBLK, B, NPIX // 128, 128).transpose(0, 3, 2, 1)
        ).reshape(NBLK * 128, -1)

    in_maps = []
    for n in range(N):
        xtn = to_slab(x[n])
        mixn = to_slab(mix[n] + bpad[:, None, None])
        in_maps.append(
            {"xt": xtn, "mixt": mixn, "wt": wt, "wtri": wtri, "ident": ident}
        )
    return in_maps


def from_slab(a):  # [(b p), (k c)] -> [C, H, W]
    return (
        a.reshape(NBLK, 128, NPIX // 128, B)
        .transpose(0, 3, 2, 1)
        .reshape(C, H, Wd)
    )


def kernel(x, mix, W, b):
    from concourse.bass_utils import run_bass_kernel_spmd

    x = np.asarray(x, np.float32)
    mix = np.asarray(mix, np.float32)
    W = np.asarray(W, np.float32)
    b = np.asarray(b, np.float32)

    nc = get_nc()
    in_maps = make_core_inputs(x, mix, W, b)
    res = run_bass_kernel_spmd(nc, in_maps, list(range(N)))

    ybar = np.empty((N, C, H, Wd), np.float32)
    mix_out = np.empty((N, C, H, Wd), np.float32)
    for n in range(N):
        ybar[n] = from_slab(res.results[n]["yt"])
        mix_out[n] = from_slab(res.results[n]["mot"])
    mix_out[:, 0] = mix[:, 0]  # reference passes mix ch0 through exactly
    return ybar, mix_out



# revision 20
# speedup vs baseline: 1.1511x; 1.1511x over previous
"""Trainium2 Bass kernel for ChannelDepsModule (sequential channel recurrence).

Math (per pixel, fp32):
    m_0 = mix_0 ; ybar_0 = round(x_0 - m_0) + m_0
    for i in 1..191:
        m_i = sum_{c<i} Wm[i-1,c] * ybar_c + b[i-1] + mix_i
        ybar_i = round(x_i - m_i) + m_i
    outputs: ybar, mix_out (= m)

Device strategy (per core, one batch image, 4096 pixels):
  - pixels on SBUF partitions ([128] x 32 chunks), channels on the free dim
  - channels in 6 blocks of 32:
      * cross-block mix contributions P via TensorE matmuls
        (stationary ybar in channel-partition layout x Wm^T slice), with
        pixel-partition PSUM output; q = x - mix - b - P is written into the
        block's ybar columns ahead of time
      * in-block recurrence: one fused DVE scan per channel computes
        t_i = q_i - sum_j w_ij y_j directly (weights negated, +1 planted on
        the diagonal so the prefilled q column enters the dot), then one
        fused DVE op assembles y_i = round(t_i) + (x_i - t_i) using the
        +-1.5*2^23 magic constant (IEEE RNE == jnp.round)
      * mix_out column m_i = x_i - t_i is produced on the GpSimd engine,
        off the critical path
      * finished ybar columns are TensorE-transposed (two half-block waves)
        into channel-partition tiles for later blocks' matmuls
  - b is folded into mix on the host; mix_out channel 0 restored on host
"""

import sys

import numpy as np

if "/opt/trn_rl_repo" not in sys.path:
    sys.path.insert(0, "/opt/trn_rl_repo")

N, C, H, Wd = 8, 192, 64, 64
NPIX = H * Wd          # 4096 pixels per core
B = 32                 # channel block size
NBLK = C // B          # 6
ROUND_C = 1.5 * 2.0**23  # fp32 add of this rounds to nearest-even integer

_CACHE = {}
_DVE_OPS = {}


def _register_dve_ops():
    """Define + register the two fused DVE ops (idempotent)."""
    if _DVE_OPS:
        return _DVE_OPS
    import concourse.dve_ops as dops
    import concourse.dve_spec as ds
    from concourse.dve_spec import AluOp, Spec, Src0, Src1
    from concourse.dve_ops import CUSTOM_DVE_SPECS, OPS, DveOp
    from concourse.dve_uop import DveOpSpec

    # The stock segmented-scan machinery only implements the page-counter
    # mode; add the documented per-page *reset* behavior for scans marked
    # with `_page_reset`: at each SUB_DIM_DONE the STEP state computes
    # d <- op(init, expr) instead of op(CURR, expr).
    if not getattr(ds, "_page_reset_patched", False):
        _orig = ds._scan_overrides

        def _patched(scans, node_stage):
            seed, step = _orig(scans, node_stage)
            for sc in scans:
                if getattr(sc, "_page_reset", False):
                    d = node_stage[sc]
                    step[d] = ds._Stage(sc.op, ds._scan_init(sc), sc.expr)
            return seed, step

        ds._scan_overrides = _patched
        ds._page_reset_patched = True

    def _chaindot_ref(in0, in1, s0, s1, imm2):
        p = in0.shape[0]
        inner = in0.shape[-1]
        a = in0.reshape(p, -1, inner).astype(np.float32)
        bb = in1.reshape(p, -1, inner).astype(np.float32)
        return np.cumsum(a * bb, axis=-1, dtype=np.float32).reshape(in0.shape)

    sc = ds.scan(AluOp.ADD, Src0 * Src1)
    object.__setattr__(sc, "_page_reset", True)
    spec_cd = Spec(body=sc, reference=_chaindot_ref)

    def _quanty_ref(in0, in1, s0, s1, imm2):
        c = np.float32(s0)
        t = in0.astype(np.float32)
        return ((t + c) - c) + (in1.astype(np.float32) - t)

    spec_qy = Spec(
        body=((Src0 + ds.C0) - ds.C0) + (Src1 - Src0), reference=_quanty_ref
    )

    def _mk(name, spec, subdim):
        if any(o.name == name for o in OPS):
            op = next(o for o in OPS if o.name == name)
        else:
            shas = {}
            for ver in ("v3", "v4"):
                shas[ver] = DveOpSpec(
                    name=name, uops=ds.lower(spec, ver=ver)
                ).sha(ver)
            op = DveOp(name, spec, subdim=subdim, uops_sha=shas)
            OPS.append(op)
            CUSTOM_DVE_SPECS[name] = spec
            dops._SUB_OPCODE_FOR_NAME[name] = dops._CUSTOM_DVE_ROW_BASE + len(OPS) - 1
        return op

    _DVE_OPS["chaindot"] = _mk("CHAINDOT_SEQ_ANT", spec_cd, subdim=True)
    _DVE_OPS["quanty"] = _mk("QUANTY_ANT", spec_qy, subdim=False)
    return _DVE_OPS


def _build(n_chunks):
    """Build + compile the per-core Bass module. n_chunks pixel chunks of 128."""
    import concourse.bacc as bacc
    import concourse.mybir as mybir
    from concourse.tile import TileContext

    ops = _register_dve_ops()
    npix = n_chunks * 128
    fp32 = mybir.dt.float32
    K = n_chunks  # pixel chunks

    nc = bacc.Bacc(None, target_bir_lowering=False)

    # dram layout: row (b, p) holds the [K, B] slab of block b, lane p —
    # 4KB contiguous runs on both sides of the DMA, and block-columns of
    # any 4 chunks are contiguous in SBUF (single-free-dim transpose APs)
    xt = nc.dram_tensor("xt", [NBLK * 128, K * B], fp32, kind="ExternalInput")
    mixt = nc.dram_tensor("mixt", [NBLK * 128, K * B], fp32, kind="ExternalInput")
    wt = nc.dram_tensor("wt", [C, C], fp32, kind="ExternalInput")
    wtri = nc.dram_tensor("wtri", [1, NBLK * B * B], fp32, kind="ExternalInput")
    ident = nc.dram_tensor("ident", [128, 128], fp32, kind="ExternalInput")
    yt = nc.dram_tensor("yt", [NBLK * 128, K * B], fp32, kind="ExternalOutput")
    mot = nc.dram_tensor("mot", [NBLK * 128, K * B], fp32, kind="ExternalOutput")

    with TileContext(nc) as tc:
        with (
            tc.tile_pool(name="big", bufs=1) as big,
            tc.tile_pool(name="small", bufs=1) as small,
            tc.tile_pool(name="scr", bufs=3) as scr,
            tc.tile_pool(name="qp", bufs=2) as qp,
            tc.tile_pool(name="psum_e", bufs=3, space="PSUM") as psum_e,
            tc.tile_pool(name="psumt", bufs=2, space="PSUM") as psumt,
        ):
            # pixel-partition tiles, free layout = k*192 + c
            X = big.tile([128, K * C], fp32, tag="X")
            MIX = big.tile([128, K * C], fp32, tag="MIX")  # becomes mix_out
            XMB = big.tile([128, K * C], fp32, tag="XMB")
            Y = big.tile([128, K * C], fp32, tag="Y")
            # channel-partition decoded ybar: chans 0-127 / 128-159
            ysb_lo = big.tile([128, npix], fp32, tag="ysb_lo")
            ysb_hi = big.tile([32, npix], fp32, tag="ysb_hi")

            wt_lo = small.tile([128, C], fp32, tag="wt_lo")
            wt_hi = small.tile([64, C], fp32, tag="wt_hi")
            wtri_t = small.tile([1, NBLK * B * B], fp32, tag="wtri")
            wtri_b = small.tile([128, NBLK * B * B], fp32, tag="wtri_b")
            id_t = small.tile([128, 128], fp32, tag="ident")

            def big_in(tile, dram):
                nc.sync.dma_start(
                    tile[:].rearrange("p (b f) -> p b f", f=K * B),
                    dram[:].rearrange("(b p) f -> p b f", p=128),
                )

            def blk_in(tile, dram, sb):
                nc.sync.dma_start(
                    tile[:, sb * K * B : (sb + 1) * K * B],
                    dram[sb * 128 : (sb + 1) * 128, :],
                )

            for sb in range(NBLK):
                blk_in(X, xt, sb)
                blk_in(MIX, mixt, sb)
            nc.sync.dma_start(wt_lo[:], wt[0:128, :])
            nc.sync.dma_start(wt_hi[:], wt[128:C, :])
            nc.sync.dma_start(wtri_t[:], wtri[:])
            nc.sync.dma_start(id_t[:], ident[:])
            nc.gpsimd.partition_broadcast(wtri_b[:], wtri_t[:])

            W = 2           # pixel waves: wave A's scan hides wave B's tail
            KW = K // W
            FB = K * B      # free-size of one block slab

            def col(tile, ch, w):  # strided [128, KW] view of channel ch
                b, c = divmod(ch, B)
                return tile[:].rearrange("p (b k c) -> p b k c", c=B, k=K)[
                    :, b, w * KW : (w + 1) * KW, c
                ]

            def ycols(sb, w, j0, j1):  # [128, KW, j1-j0] view of block cols
                return Y[:].rearrange("p (b k c) -> p b k c", c=B, k=K)[
                    :, sb, w * KW : (w + 1) * KW, j0:j1
                ]

            def xmb_slice(sb, w):
                return XMB[:].rearrange("p (b k c) -> p b k c", c=B, k=K)[
                    :, sb, w * KW : (w + 1) * KW, :
                ]

            def xmb_prep(sb):  # XMB = X - (MIX + b) for one block slab
                s = slice(sb * FB, (sb + 1) * FB)
                nc.vector.tensor_sub(XMB[:, s], X[:, s], MIX[:, s])

            def p_all(sb, w, pp):
                """P for block sb, wave w: one matmul per chunk over all
                decoded chans (contract 0..32*sb; +32-row tail for sb=5)."""
                kdec = min(sb * B, 128)
                for kk in range(KW):
                    k = w * KW + kk
                    nc.tensor.matmul(
                        pp[:, kk * B : (kk + 1) * B],
                        ysb_lo[0:kdec, k * 128 : (k + 1) * 128],
                        wt_lo[0:kdec, sb * B : (sb + 1) * B],
                        start=(kk == 0),
                        stop=(sb * B <= 128) and (kk == KW - 1),
                    )
                    if sb * B > 128:
                        nc.tensor.matmul(
                            pp[:, kk * B : (kk + 1) * B],
                            ysb_hi[0:B, k * 128 : (k + 1) * 128],
                            wt_hi[0:B, sb * B : (sb + 1) * B],
                            start=False,
                            stop=(kk == KW - 1),
                        )

            def q_fin(sb, w, pp):
                """Block sb's Y cols = XMB - PP."""
                nc.vector.tensor_sub(
                    ycols(sb, w, 0, B),
                    xmb_slice(sb, w),
                    pp[:].rearrange("p (k c) -> p k c", c=B),
                )

            def transpose_block(sb, w):
                """Transpose Y cols of block sb, wave w into ysb (chan-part).

                4 pixel-chunks per TensorE transpose ([128,128] stationary,
                one identity stream); the chunk-interleaved PSUM result is
                de-interleaved by four [32,128] ScalarE copies.
                """
                base = sb * B
                if base < 128:
                    dst, dr0 = ysb_lo, base
                else:
                    dst, dr0 = ysb_hi, base - 128
                slab = Y[:].rearrange("p (b f) -> p b f", f=FB)[:, sb, :]
                for g in range(w * KW, w * KW + KW, 4):
                    pt = psumt.tile([128, 128], fp32, tag="pt")
                    nc.tensor.transpose(
                        pt[:],
                        slab[:, g * B : (g + 4) * B],
                        id_t[:],
                    )
                    for j in range(4):
                        nc.scalar.copy(
                            dst[dr0 : dr0 + B, (g + j) * 128 : (g + j + 1) * 128],
                            pt[j * B : (j + 1) * B, :],
                        )

            def steps(sb, w):
                base = sb * B
                for i in range(B):
                    ch = base + i
                    if i > 0:
                        prod = scr.tile([128, B * KW], fp32, tag="prod")
                        pr = prod[:].rearrange("p (k c) -> p k c", c=B)[
                            :, :, 0 : i + 1
                        ]
                        woff = sb * B * B + i * B
                        wrow = (
                            wtri_b[:, woff : woff + i + 1]
                            .unsqueeze(1)
                            .broadcast_to([128, KW, i + 1])
                        )
                        nc.vector._custom_dve(
                            ops["chaindot"], out=pr,
                            in0=ycols(sb, w, 0, i + 1), in1=wrow,
                        )
                        t_ap = prod[:].rearrange("p (k c) -> p k c", c=B)[
                            :, :, i
                        ]
                    else:
                        t_ap = col(Y, ch, w)
                    # mix_out column (off critical path, on GpSimd)
                    nc.gpsimd.tensor_sub(col(MIX, ch, w), col(X, ch, w), t_ap)
                    # y = round(t) + (x - t)
                    nc.vector._custom_dve(
                        ops["quanty"], out=col(Y, ch, w), in0=t_ap,
                        in1=col(X, ch, w), s0=ROUND_C,
                    )

            def blk_out(dram, tile, sb):
                nc.sync.dma_start(
                    dram[sb * 128 : (sb + 1) * 128, :],
                    tile[:, sb * FB : (sb + 1) * FB],
                )

            # ---------------- schedule ----------------
            # Wave pipeline: wave w's scan overlaps the other wave's
            # transpose -> p_all -> q_fin tail on Tensor/Scalar.
            pp_cur = [None] * W
            for sb in range(NBLK):
                xmb_prep(sb)
                for w in range(W):
                    if sb == 0:
                        nc.vector.tensor_copy(
                            ycols(sb, w, 0, B), xmb_slice(sb, w)
                        )
                    else:
                        q_fin(sb, w, pp_cur[w])
                    steps(sb, w)
                    if sb + 1 < NBLK:
                        transpose_block(sb, w)
                        pp = psum_e.tile([128, B * KW], fp32, tag="pp")
                        p_all(sb + 1, w, pp)
                        pp_cur[w] = pp
                blk_out(yt, Y, sb)
                blk_out(mot, MIX, sb)

    nc.compile()
    return nc


def get_nc(n_chunks=NPIX // 128):
    if n_chunks not in _CACHE:
        _CACHE[n_chunks] = _build(n_chunks)
    return _CACHE[n_chunks]


def make_core_inputs(x, mix, W, b):
    """Host-side layout prep. Returns list of per-core input dicts."""
    Wm = (W * np.tril(np.ones((C - 1, C), np.float32))).astype(np.float32)
    wt = np.zeros((C, C), np.float32)
    wt[:, 1:] = Wm.T  # wt[c, i] = Wm[i-1, c]
    # in-block triangle, negated, with +1 on the diagonal: the scan over
    # [y_0..y_{i-1}, q_i] then yields t_i = q_i - sum_j w_ij y_j directly
    wtri = np.zeros((NBLK, B, B), np.float32)
    for sb in range(NBLK):
        for i in range(1, B):
            ch = sb * B + i
            wtri[sb, i, :i] = -Wm[ch - 1, sb * B : sb * B + i]
            wtri[sb, i, i] = 1.0
    wtri = wtri.reshape(1, -1)
    bpad = np.zeros((C,), np.float32)
    bpad[1:] = b
    ident = np.eye(128, dtype=np.float32)

    def to_slab(a):  # [C, H, W] -> [(b p), (k c)]
        return np.ascontiguousarray(
            a.reshape(NBLK, B, NPIX // 128, 128).transpose(0, 3, 2, 1)
        ).reshape(NBLK * 128, -1)

    in_maps = []
    for n in range(N):
        xtn = to_slab(x[n])
        mixn = to_slab(mix[n] + bpad[:, None, None])
        in_maps.append(
            {"xt": xtn, "mixt": mixn, "wt": wt, "wtri": wtri, "ident": ident}
        )
    return in_maps


def from_slab(a):  # [(b p), (k c)] -> [C, H, W]
    return (
        a.reshape(NBLK, 128, NPIX // 128, B)
        .transpose(0, 3, 2, 1)
        .reshape(C, H, Wd)
    )


def kernel(x, mix, W, b):
    from concourse.bass_utils import run_bass_kernel_spmd

    x = np.asarray(x, np.float32)
    mix = np.asarray(mix, np.float32)
    W = np.asarray(W, np.float32)
    b = np.asarray(b, np.float32)

    nc = get_nc()
    in_maps = make_core_inputs(x, mix, W, b)
    res = run_bass_kernel_spmd(nc, in_maps, list(range(N)))

    ybar = np.empty((N, C, H, Wd), np.float32)
    mix_out = np.empty((N, C, H, Wd), np.float32)
    for n in range(N):
        ybar[n] = from_slab(res.results[n]["yt"])
        mix_out[n] = from_slab(res.results[n]["mot"])
    mix_out[:, 0] = mix[:, 0]  # reference passes mix ch0 through exactly
    return ybar, mix_out



# revision 28
# speedup vs baseline: 1.1731x; 1.0191x over previous
"""Trainium2 Bass kernel for ChannelDepsModule (sequential channel recurrence).

Math (per pixel, fp32):
    m_0 = mix_0 ; ybar_0 = round(x_0 - m_0) + m_0
    for i in 1..191:
        m_i = sum_{c<i} Wm[i-1,c] * ybar_c + b[i-1] + mix_i
        ybar_i = round(x_i - m_i) + m_i
    outputs: ybar, mix_out (= m)

Device strategy (per core, one batch image, 4096 pixels):
  - pixels on SBUF partitions ([128] x 32 chunks), channels on the free
    dim; SBUF free layout is (block, chunk, channel) so one block's
    4-chunk column group is contiguous (single-free-dim transpose APs,
    4KB-run DMAs via host-side slab permutation)
  - pixel chunks split into 2 waves of 16; the wave pipeline overlaps
    wave A's sequential DVE scan with wave B's TensorE/ScalarE tail
    (transpose -> de-interleave -> next block's P matmuls -> q prefill)
  - channels in 6 blocks of 32:
      * cross-block mix contributions P via one TensorE matmul per
        chunk (stationary ybar in channel-partition layout x Wm^T
        slice, contract = all decoded channels), pixel-partition PSUM out;
        q = x - mix - b - P is written into the block's ybar columns
      * in-block recurrence: one fused DVE scan per channel computes
        t_i = q_i - sum_j w_ij y_j directly (weights negated, +1 planted on
        the diagonal so the prefilled q column enters the dot), then one
        fused DVE op assembles y_i = round(t_i) + (x_i - t_i) using the
        +-1.5*2^23 magic constant (IEEE RNE == jnp.round)
      * mix_out column m_i = x_i - t_i is produced on the GpSimd engine,
        off the critical path
      * finished ybar columns are TensorE-transposed 4 chunks per
        instruction into chunk-interleaved PSUM, then de-interleaved into
        channel-partition tiles by [32,128] ScalarE/GpSimd copies
  - b is folded into mix on the host; mix_out channel 0 restored on host
"""

import sys

import numpy as np

if "/opt/trn_rl_repo" not in sys.path:
    sys.path.insert(0, "/opt/trn_rl_repo")

N, C, H, Wd = 8, 192, 64, 64
NPIX = H * Wd          # 4096 pixels per core
B = 32                 # channel block size
NBLK = C // B          # 6
ROUND_C = 1.5 * 2.0**23  # fp32 add of this rounds to nearest-even integer

_CACHE = {}
_DVE_OPS = {}


def _register_dve_ops():
    """Define + register the two fused DVE ops (idempotent)."""
    if _DVE_OPS:
        return _DVE_OPS
    import concourse.dve_ops as dops
    import concourse.dve_spec as ds
    from concourse.dve_spec import AluOp, Spec, Src0, Src1
    from concourse.dve_ops import CUSTOM_DVE_SPECS, OPS, DveOp
    from concourse.dve_uop import DveOpSpec

    # The stock segmented-scan machinery only implements the page-counter
    # mode; add the documented per-page *reset* behavior for scans marked
    # with `_page_reset`: at each SUB_DIM_DONE the STEP state computes
    # d <- op(init, expr) instead of op(CURR, expr).
    if not getattr(ds, "_page_reset_patched", False):
        _orig = ds._scan_overrides

        def _patched(scans, node_stage):
            seed, step = _orig(scans, node_stage)
            for sc in scans:
                if getattr(sc, "_page_reset", False):
                    d = node_stage[sc]
                    step[d] = ds._Stage(sc.op, ds._scan_init(sc), sc.expr)
            return seed, step

        ds._scan_overrides = _patched
        ds._page_reset_patched = True

    def _chaindot_ref(in0, in1, s0, s1, imm2):
        p = in0.shape[0]
        inner = in0.shape[-1]
        a = in0.reshape(p, -1, inner).astype(np.float32)
        bb = in1.reshape(p, -1, inner).astype(np.float32)
        return np.cumsum(a * bb, axis=-1, dtype=np.float32).reshape(in0.shape)

    sc = ds.scan(AluOp.ADD, Src0 * Src1)
    object.__setattr__(sc, "_page_reset", True)
    spec_cd = Spec(body=sc, reference=_chaindot_ref)

    def _quanty_ref(in0, in1, s0, s1, imm2):
        c = np.float32(s0)
        t = in0.astype(np.float32)
        return ((t + c) - c) + (in1.astype(np.float32) - t)

    spec_qy = Spec(
        body=((Src0 + ds.C0) - ds.C0) + (Src1 - Src0), reference=_quanty_ref
    )

    def _mk(name, spec, subdim):
        if any(o.name == name for o in OPS):
            op = next(o for o in OPS if o.name == name)
        else:
            shas = {}
            for ver in ("v3", "v4"):
                shas[ver] = DveOpSpec(
                    name=name, uops=ds.lower(spec, ver=ver)
                ).sha(ver)
            op = DveOp(name, spec, subdim=subdim, uops_sha=shas)
            OPS.append(op)
            CUSTOM_DVE_SPECS[name] = spec
            dops._SUB_OPCODE_FOR_NAME[name] = dops._CUSTOM_DVE_ROW_BASE + len(OPS) - 1
        return op

    _DVE_OPS["chaindot"] = _mk("CHAINDOT_SEQ_ANT", spec_cd, subdim=True)
    _DVE_OPS["quanty"] = _mk("QUANTY_ANT", spec_qy, subdim=False)
    return _DVE_OPS


def _build(n_chunks):
    """Build + compile the per-core Bass module. n_chunks pixel chunks of 128."""
    import concourse.bacc as bacc
    import concourse.mybir as mybir
    from concourse.tile import TileContext

    ops = _register_dve_ops()
    npix = n_chunks * 128
    fp32 = mybir.dt.float32
    K = n_chunks  # pixel chunks

    nc = bacc.Bacc(None, target_bir_lowering=False)

    # dram layout: row (b, p) holds the [K, B] slab of block b, lane p —
    # 4KB contiguous runs on both sides of the DMA, and block-columns of
    # any 4 chunks are contiguous in SBUF (single-free-dim transpose APs)
    xt = nc.dram_tensor("xt", [NBLK * 128, K * B], fp32, kind="ExternalInput")
    mixt = nc.dram_tensor("mixt", [NBLK * 128, K * B], fp32, kind="ExternalInput")
    wt = nc.dram_tensor("wt", [C, C], fp32, kind="ExternalInput")
    wtri = nc.dram_tensor(
        "wtri", [128, NBLK * B * B], fp32, kind="ExternalInput"
    )
    ident = nc.dram_tensor("ident", [128, 128], fp32, kind="ExternalInput")
    yt = nc.dram_tensor("yt", [NBLK * 128, K * B], fp32, kind="ExternalOutput")
    mot = nc.dram_tensor("mot", [NBLK * 128, K * B], fp32, kind="ExternalOutput")

    with TileContext(nc) as tc:
        with (
            tc.tile_pool(name="big", bufs=1) as big,
            tc.tile_pool(name="small", bufs=1) as small,
            tc.tile_pool(name="scr", bufs=3) as scr,
            tc.tile_pool(name="qp", bufs=2) as qp,
            tc.tile_pool(name="psum_e", bufs=3, space="PSUM") as psum_e,
            tc.tile_pool(name="psumt", bufs=2, space="PSUM") as psumt,
        ):
            # pixel-partition tiles, free layout = k*192 + c
            X = big.tile([128, K * C], fp32, tag="X")
            MIX = big.tile([128, K * C], fp32, tag="MIX")  # becomes mix_out
            XMB = big.tile([128, K * C], fp32, tag="XMB")
            Y = big.tile([128, K * C], fp32, tag="Y")
            # channel-partition decoded ybar: chans 0-127 / 128-159
            ysb_lo = big.tile([128, npix], fp32, tag="ysb_lo")
            ysb_hi = big.tile([32, npix], fp32, tag="ysb_hi")

            wt_lo = small.tile([128, C], fp32, tag="wt_lo")
            wt_hi = small.tile([64, C], fp32, tag="wt_hi")
            wtri_b = small.tile([128, NBLK * B * B], fp32, tag="wtri_b")
            id_t = small.tile([128, 128], fp32, tag="ident")

            def big_in(tile, dram):
                nc.sync.dma_start(
                    tile[:].rearrange("p (b f) -> p b f", f=K * B),
                    dram[:].rearrange("(b p) f -> p b f", p=128),
                )

            def blk_in(tile, dram, sb):
                nc.sync.dma_start(
                    tile[:, sb * K * B : (sb + 1) * K * B],
                    dram[sb * 128 : (sb + 1) * 128, :],
                )

            for sb in range(NBLK):
                blk_in(X, xt, sb)
                blk_in(MIX, mixt, sb)
            nc.sync.dma_start(wt_lo[:], wt[0:128, :])
            nc.sync.dma_start(wt_hi[:], wt[128:C, :])
            nc.sync.dma_start(wtri_b[:], wtri[:])
            nc.sync.dma_start(id_t[:], ident[:])

            W = 2           # pixel waves: wave A's scan hides wave B's tail
            KW = K // W
            FB = K * B      # free-size of one block slab

            def col(tile, ch, w):  # strided [128, KW] view of channel ch
                b, c = divmod(ch, B)
                return tile[:].rearrange("p (b k c) -> p b k c", c=B, k=K)[
                    :, b, w * KW : (w + 1) * KW, c
                ]

            def ycols(sb, w, j0, j1):  # [128, KW, j1-j0] view of block cols
                return Y[:].rearrange("p (b k c) -> p b k c", c=B, k=K)[
                    :, sb, w * KW : (w + 1) * KW, j0:j1
                ]

            def xmb_slice(sb, w):
                return XMB[:].rearrange("p (b k c) -> p b k c", c=B, k=K)[
                    :, sb, w * KW : (w + 1) * KW, :
                ]

            def xmb_prep(sb):  # XMB = X - (MIX + b) for one block slab
                s = slice(sb * FB, (sb + 1) * FB)
                nc.vector.tensor_sub(XMB[:, s], X[:, s], MIX[:, s])

            def p_all(sb, w, pp):
                """P for block sb, wave w: one matmul per chunk over all
                decoded chans (contract 0..32*sb; +32-row tail for sb=5)."""
                kdec = min(sb * B, 128)
                for kk in range(KW):
                    k = w * KW + kk
                    nc.tensor.matmul(
                        pp[:, kk * B : (kk + 1) * B],
                        ysb_lo[0:kdec, k * 128 : (k + 1) * 128],
                        wt_lo[0:kdec, sb * B : (sb + 1) * B],
                        start=(kk == 0),
                        stop=(sb * B <= 128) and (kk == KW - 1),
                    )
                    if sb * B > 128:
                        nc.tensor.matmul(
                            pp[:, kk * B : (kk + 1) * B],
                            ysb_hi[0:B, k * 128 : (k + 1) * 128],
                            wt_hi[0:B, sb * B : (sb + 1) * B],
                            start=False,
                            stop=(kk == KW - 1),
                        )

            def q_fin(sb, w, pp):
                """Block sb's Y cols = XMB - PP."""
                nc.vector.tensor_sub(
                    ycols(sb, w, 0, B),
                    xmb_slice(sb, w),
                    pp[:].rearrange("p (k c) -> p k c", c=B),
                )

            def transpose_block(sb, w):
                """Transpose Y cols of block sb, wave w into ysb (chan-part).

                4 pixel-chunks per TensorE transpose ([128,128] stationary,
                one identity stream); the chunk-interleaved PSUM result is
                de-interleaved by four [32,128] ScalarE copies.
                """
                base = sb * B
                if base < 128:
                    dst, dr0 = ysb_lo, base
                else:
                    dst, dr0 = ysb_hi, base - 128
                slab = Y[:].rearrange("p (b f) -> p b f", f=FB)[:, sb, :]
                for g in range(w * KW, w * KW + KW, 4):
                    pt = psumt.tile([128, 128], fp32, tag="pt")
                    nc.tensor.transpose(
                        pt[:],
                        slab[:, g * B : (g + 4) * B],
                        id_t[:],
                    )
                    for j in range(4):
                        nc.scalar.copy(
                            dst[dr0 : dr0 + B, (g + j) * 128 : (g + j + 1) * 128],
                            pt[j * B : (j + 1) * B, :],
                        )

            def steps(sb, w):
                base = sb * B
                for i in range(B):
                    ch = base + i
                    if i > 0:
                        prod = scr.tile([128, B * KW], fp32, tag="prod")
                        pr = prod[:].rearrange("p (k c) -> p k c", c=B)[
                            :, :, 0 : i + 1
                        ]
                        woff = sb * B * B + i * B
                        wrow = (
                            wtri_b[:, woff : woff + i + 1]
                            .unsqueeze(1)
                            .broadcast_to([128, KW, i + 1])
                        )
                        nc.vector._custom_dve(
                            ops["chaindot"], out=pr,
                            in0=ycols(sb, w, 0, i + 1), in1=wrow,
                        )
                        t_ap = prod[:].rearrange("p (k c) -> p k c", c=B)[
                            :, :, i
                        ]
                    else:
                        t_ap = col(Y, ch, w)
                    # mix_out column (off critical path, on GpSimd)
                    nc.gpsimd.tensor_sub(col(MIX, ch, w), col(X, ch, w), t_ap)
                    # y = round(t) + (x - t)
                    nc.vector._custom_dve(
                        ops["quanty"], out=col(Y, ch, w), in0=t_ap,
                        in1=col(X, ch, w), s0=ROUND_C,
                    )

            def blk_out(dram, tile, sb):
                nc.sync.dma_start(
                    dram[sb * 128 : (sb + 1) * 128, :],
                    tile[:, sb * FB : (sb + 1) * FB],
                )

            # ---------------- schedule ----------------
            # Wave pipeline: wave w's scan overlaps the other wave's
            # transpose -> p_all -> q_fin tail on Tensor/Scalar.
            pp_cur = [None] * W
            for sb in range(NBLK):
                xmb_prep(sb)
                for w in range(W):
                    if sb == 0:
                        nc.vector.tensor_copy(
                            ycols(sb, w, 0, B), xmb_slice(sb, w)
                        )
                    else:
                        q_fin(sb, w, pp_cur[w])
                    steps(sb, w)
                    if sb + 1 < NBLK:
                        transpose_block(sb, w)
                        pp = psum_e.tile([128, B * KW], fp32, tag="pp")
                        p_all(sb + 1, w, pp)
                        pp_cur[w] = pp
                blk_out(yt, Y, sb)
                blk_out(mot, MIX, sb)

    nc.compile()
    return nc


def get_nc(n_chunks=NPIX // 128):
    if n_chunks not in _CACHE:
        _CACHE[n_chunks] = _build(n_chunks)
    return _CACHE[n_chunks]


def make_core_inputs(x, mix, W, b):
    """Host-side layout prep. Returns list of per-core input dicts."""
    Wm = (W * np.tril(np.ones((C - 1, C), np.float32))).astype(np.float32)
    wt = np.zeros((C, C), np.float32)
    wt[:, 1:] = Wm.T  # wt[c, i] = Wm[i-1, c]
    # in-block triangle, negated, with +1 on the diagonal: the scan over
    # [y_0..y_{i-1}, q_i] then yields t_i = q_i - sum_j w_ij y_j directly
    wtri = np.zeros((NBLK, B, B), np.float32)
    for sb in range(NBLK):
        for i in range(1, B):
            ch = sb * B + i
            wtri[sb, i, :i] = -Wm[ch - 1, sb * B : sb * B + i]
            wtri[sb, i, i] = 1.0
    wtri = np.ascontiguousarray(
        np.broadcast_to(wtri.reshape(1, -1), (128, NBLK * B * B))
    )
    bpad = np.zeros((C,), np.float32)
    bpad[1:] = b
    ident = np.eye(128, dtype=np.float32)

    def to_slab(a):  # [C, H, W] -> [(b p), (k c)]
        return np.ascontiguousarray(
            a.reshape(NBLK, B, NPIX // 128, 128).transpose(0, 3, 2, 1)
        ).reshape(NBLK * 128, -1)

    in_maps = []
    for n in range(N):
        xtn = to_slab(x[n])
        mixn = to_slab(mix[n] + bpad[:, None, None])
        in_maps.append(
            {"xt": xtn, "mixt": mixn, "wt": wt, "wtri": wtri, "ident": ident}
        )
    return in_maps


def from_slab(a):  # [(b p), (k c)] -> [C, H, W]
    return (
        a.reshape(NBLK, 128, NPIX // 128, B)
        .transpose(0, 3, 2, 1)
        .reshape(C, H, Wd)
    )


def kernel(x, mix, W, b):
    from concourse.bass_utils import run_bass_kernel_spmd

    x = np.asarray(x, np.float32)
    mix = np.asarray(mix, np.float32)
    W = np.asarray(W, np.float32)
    b = np.asarray(b, np.float32)

    nc = get_nc()
    in_maps = make_core_inputs(x, mix, W, b)
    res = run_bass_kernel_spmd(nc, in_maps, list(range(N)))

    ybar = np.empty((N, C, H, Wd), np.float32)
    mix_out = np.empty((N, C, H, Wd), np.float32)
    for n in range(N):
        ybar[n] = from_slab(res.results[n]["yt"])
        mix_out[n] = from_slab(res.results[n]["mot"])
    mix_out[:, 0] = mix[:, 0]  # reference passes mix ch0 through exactly
    return ybar, mix_out



# revision 30
# speedup vs baseline: 1.2010x; 1.0238x over previous
"""Trainium2 Bass kernel for ChannelDepsModule (sequential channel recurrence).

Math (per pixel, fp32):
    m_0 = mix_0 ; ybar_0 = round(x_0 - m_0) + m_0
    for i in 1..191:
        m_i = sum_{c<i} Wm[i-1,c] * ybar_c + b[i-1] + mix_i
        ybar_i = round(x_i - m_i) + m_i
    outputs: ybar, mix_out (= m)

Device strategy (per core, one batch image, 4096 pixels):
  - pixels on SBUF partitions ([128] x 32 chunks), channels on the free
    dim; SBUF free layout is (block, chunk, channel) so one block's
    4-chunk column group is contiguous (single-free-dim transpose APs,
    4KB-run DMAs via host-side slab permutation)
  - pixel chunks split into 2 waves of 16; the wave pipeline overlaps
    wave A's sequential DVE scan with wave B's TensorE/ScalarE tail
    (transpose -> de-interleave -> next block's P matmuls -> q prefill)
  - channels in 6 blocks of 32:
      * cross-block mix contributions P via one TensorE matmul per
        chunk (stationary ybar in channel-partition layout x Wm^T
        slice, contract = all decoded channels), pixel-partition PSUM out;
        q = x - mix - b - P is written into the block's ybar columns
      * in-block recurrence: one fused DVE scan per channel computes
        t_i = q_i - sum_j w_ij y_j directly (weights negated, +1 planted on
        the diagonal so the prefilled q column enters the dot), then one
        fused DVE op assembles y_i = round(t_i) + (x_i - t_i) using the
        +-1.5*2^23 magic constant (IEEE RNE == jnp.round)
      * mix_out column m_i = x_i - t_i is produced on the GpSimd engine,
        off the critical path
      * finished ybar columns are TensorE-transposed 4 chunks per
        instruction into chunk-interleaved PSUM, then de-interleaved into
        channel-partition tiles by [32,128] ScalarE/GpSimd copies
  - b is folded into mix on the host; mix_out channel 0 restored on host
"""

import sys

import numpy as np

if "/opt/trn_rl_repo" not in sys.path:
    sys.path.insert(0, "/opt/trn_rl_repo")

N, C, H, Wd = 8, 192, 64, 64
NPIX = H * Wd          # 4096 pixels per core
B = 32                 # channel block size
NBLK = C // B          # 6
ROUND_C = 1.5 * 2.0**23  # fp32 add of this rounds to nearest-even integer

_CACHE = {}
_DVE_OPS = {}


def _register_dve_ops():
    """Define + register the two fused DVE ops (idempotent)."""
    if _DVE_OPS:
        return _DVE_OPS
    import concourse.dve_ops as dops
    import concourse.dve_spec as ds
    from concourse.dve_spec import AluOp, Spec, Src0, Src1
    from concourse.dve_ops import CUSTOM_DVE_SPECS, OPS, DveOp
    from concourse.dve_uop import DveOpSpec

    # The stock segmented-scan machinery only implements the page-counter
    # mode; add the documented per-page *reset* behavior for scans marked
    # with `_page_reset`: at each SUB_DIM_DONE the STEP state computes
    # d <- op(init, expr) instead of op(CURR, expr).
    if not getattr(ds, "_page_reset_patched", False):
        _orig = ds._scan_overrides

        def _patched(scans, node_stage):
            seed, step = _orig(scans, node_stage)
            for sc in scans:
                if getattr(sc, "_page_reset", False):
                    d = node_stage[sc]
                    step[d] = ds._Stage(sc.op, ds._scan_init(sc), sc.expr)
            return seed, step

        ds._scan_overrides = _patched
        ds._page_reset_patched = True

    def _chaindot_ref(in0, in1, s0, s1, imm2):
        p = in0.shape[0]
        inner = in0.shape[-1]
        a = in0.reshape(p, -1, inner).astype(np.float32)
        bb = in1.reshape(p, -1, inner).astype(np.float32)
        return np.cumsum(a * bb, axis=-1, dtype=np.float32).reshape(in0.shape)

    sc = ds.scan(AluOp.ADD, Src0 * Src1)
    object.__setattr__(sc, "_page_reset", True)
    spec_cd = Spec(body=sc, reference=_chaindot_ref)

    def _quanty_ref(in0, in1, s0, s1, imm2):
        c = np.float32(s0)
        t = in0.astype(np.float32)
        return ((t + c) - c) + (in1.astype(np.float32) - t)

    spec_qy = Spec(
        body=((Src0 + ds.C0) - ds.C0) + (Src1 - Src0), reference=_quanty_ref
    )

    def _mk(name, spec, subdim):
        if any(o.name == name for o in OPS):
            op = next(o for o in OPS if o.name == name)
        else:
            shas = {}
            for ver in ("v3", "v4"):
                shas[ver] = DveOpSpec(
                    name=name, uops=ds.lower(spec, ver=ver)
                ).sha(ver)
            op = DveOp(name, spec, subdim=subdim, uops_sha=shas)
            OPS.append(op)
            CUSTOM_DVE_SPECS[name] = spec
            dops._SUB_OPCODE_FOR_NAME[name] = dops._CUSTOM_DVE_ROW_BASE + len(OPS) - 1
        return op

    _DVE_OPS["chaindot"] = _mk("CHAINDOT_SEQ_ANT", spec_cd, subdim=True)
    _DVE_OPS["quanty"] = _mk("QUANTY_ANT", spec_qy, subdim=False)
    return _DVE_OPS


def _build(n_chunks):
    """Build + compile the per-core Bass module. n_chunks pixel chunks of 128."""
    import concourse.bacc as bacc
    import concourse.mybir as mybir
    from concourse.tile import TileContext

    ops = _register_dve_ops()
    npix = n_chunks * 128
    fp32 = mybir.dt.float32
    K = n_chunks  # pixel chunks

    nc = bacc.Bacc(None, target_bir_lowering=False)

    # dram layout: row (b, p) holds the [K, B] slab of block b, lane p —
    # 4KB contiguous runs on both sides of the DMA, and block-columns of
    # any 4 chunks are contiguous in SBUF (single-free-dim transpose APs)
    xt = nc.dram_tensor("xt", [NBLK * 128, K * B], fp32, kind="ExternalInput")
    mixt = nc.dram_tensor("mixt", [NBLK * 128, K * B], fp32, kind="ExternalInput")
    wt = nc.dram_tensor("wt", [C, C], fp32, kind="ExternalInput")
    wtri = nc.dram_tensor(
        "wtri", [128, NBLK * B * B], fp32, kind="ExternalInput"
    )
    ident = nc.dram_tensor("ident", [128, 128], fp32, kind="ExternalInput")
    yt = nc.dram_tensor("yt", [NBLK * 128, K * B], fp32, kind="ExternalOutput")
    mot = nc.dram_tensor("mot", [NBLK * 128, K * B], fp32, kind="ExternalOutput")

    with TileContext(nc) as tc:
        with (
            tc.tile_pool(name="big", bufs=1) as big,
            tc.tile_pool(name="small", bufs=1) as small,
            tc.tile_pool(name="scr", bufs=3) as scr,
            tc.tile_pool(name="qp", bufs=2) as qp,
            tc.tile_pool(name="psum_e", bufs=3, space="PSUM") as psum_e,
            tc.tile_pool(name="psumt", bufs=2, space="PSUM") as psumt,
        ):
            # pixel-partition tiles, free layout = k*192 + c
            X = big.tile([128, K * C], fp32, tag="X")
            MIX = big.tile([128, K * C], fp32, tag="MIX")  # becomes mix_out
            XMB = big.tile([128, K * C], fp32, tag="XMB")
            Y = big.tile([128, K * C], fp32, tag="Y")
            # channel-partition decoded ybar: chans 0-127 / 128-159
            ysb_lo = big.tile([128, npix], fp32, tag="ysb_lo")
            ysb_hi = big.tile([32, npix], fp32, tag="ysb_hi")

            wt_lo = small.tile([128, C], fp32, tag="wt_lo")
            wt_hi = small.tile([64, C], fp32, tag="wt_hi")
            wtri_b = small.tile([128, NBLK * B * B], fp32, tag="wtri_b")
            id_t = small.tile([128, 128], fp32, tag="ident")

            def big_in(tile, dram):
                nc.sync.dma_start(
                    tile[:].rearrange("p (b f) -> p b f", f=K * B),
                    dram[:].rearrange("(b p) f -> p b f", p=128),
                )

            def blk_in(tile, dram, sb):
                nc.sync.dma_start(
                    tile[:, sb * K * B : (sb + 1) * K * B],
                    dram[sb * 128 : (sb + 1) * 128, :],
                )

            for sb in range(NBLK):
                blk_in(X, xt, sb)
                blk_in(MIX, mixt, sb)
            nc.sync.dma_start(wt_lo[:], wt[0:128, :])
            nc.sync.dma_start(wt_hi[:], wt[128:C, :])
            nc.sync.dma_start(wtri_b[:], wtri[:])
            nc.sync.dma_start(id_t[:], ident[:])

            W = 2           # pixel waves: wave A's scan hides wave B's tail
            KW = K // W
            FB = K * B      # free-size of one block slab

            def col(tile, ch, w):  # strided [128, KW] view of channel ch
                b, c = divmod(ch, B)
                return tile[:].rearrange("p (b k c) -> p b k c", c=B, k=K)[
                    :, b, w * KW : (w + 1) * KW, c
                ]

            def ycols(sb, w, j0, j1):  # [128, KW, j1-j0] view of block cols
                return Y[:].rearrange("p (b k c) -> p b k c", c=B, k=K)[
                    :, sb, w * KW : (w + 1) * KW, j0:j1
                ]

            def xmb_slice(sb, w):
                return XMB[:].rearrange("p (b k c) -> p b k c", c=B, k=K)[
                    :, sb, w * KW : (w + 1) * KW, :
                ]

            def xmb_prep(sb):  # XMB = X - (MIX + b) for one block slab
                s = slice(sb * FB, (sb + 1) * FB)
                nc.vector.tensor_sub(XMB[:, s], X[:, s], MIX[:, s])

            def p_lo(sb, w, pp):
                """Contract over chans 0..min(32*sb,128) for block sb. For
                sb=5 this part skips block 4's rows, so it can be issued
                before block 4's transpose and overlap the running scan."""
                kdec = min(sb * B, 128)
                for kk in range(KW):
                    k = w * KW + kk
                    nc.tensor.matmul(
                        pp[:, kk * B : (kk + 1) * B],
                        ysb_lo[0:kdec, k * 128 : (k + 1) * 128],
                        wt_lo[0:kdec, sb * B : (sb + 1) * B],
                        start=(kk == 0),
                        stop=(sb * B <= 128) and (kk == KW - 1),
                    )

            def p_hi(sb, w, pp):
                """Remaining 32-row tail (chans 128..159) for sb=5."""
                for kk in range(KW):
                    k = w * KW + kk
                    nc.tensor.matmul(
                        pp[:, kk * B : (kk + 1) * B],
                        ysb_hi[0:B, k * 128 : (k + 1) * 128],
                        wt_hi[0:B, sb * B : (sb + 1) * B],
                        start=False,
                        stop=(kk == KW - 1),
                    )

            def q_fin(sb, w, pp):
                """Block sb's Y cols = XMB - PP."""
                nc.vector.tensor_sub(
                    ycols(sb, w, 0, B),
                    xmb_slice(sb, w),
                    pp[:].rearrange("p (k c) -> p k c", c=B),
                )

            def transpose_block(sb, w):
                """Transpose Y cols of block sb, wave w into ysb (chan-part).

                4 pixel-chunks per TensorE transpose ([128,128] stationary,
                one identity stream); the chunk-interleaved PSUM result is
                de-interleaved by four [32,128] ScalarE copies.
                """
                base = sb * B
                if base < 128:
                    dst, dr0 = ysb_lo, base
                else:
                    dst, dr0 = ysb_hi, base - 128
                slab = Y[:].rearrange("p (b f) -> p b f", f=FB)[:, sb, :]
                for g in range(w * KW, w * KW + KW, 4):
                    pt = psumt.tile([128, 128], fp32, tag="pt")
                    nc.tensor.transpose(
                        pt[:],
                        slab[:, g * B : (g + 4) * B],
                        id_t[:],
                    )
                    for j in range(4):
                        nc.scalar.copy(
                            dst[dr0 : dr0 + B, (g + j) * 128 : (g + j + 1) * 128],
                            pt[j * B : (j + 1) * B, :],
                        )

            def steps(sb, w):
                base = sb * B
                for i in range(B):
                    ch = base + i
                    if i > 0:
                        prod = scr.tile([128, B * KW], fp32, tag="prod")
                        pr = prod[:].rearrange("p (k c) -> p k c", c=B)[
                            :, :, 0 : i + 1
                        ]
                        woff = sb * B * B + i * B
                        wrow = (
                            wtri_b[:, woff : woff + i + 1]
                            .unsqueeze(1)
                            .broadcast_to([128, KW, i + 1])
                        )
                        nc.vector._custom_dve(
                            ops["chaindot"], out=pr,
                            in0=ycols(sb, w, 0, i + 1), in1=wrow,
                        )
                        t_ap = prod[:].rearrange("p (k c) -> p k c", c=B)[
                            :, :, i
                        ]
                    else:
                        t_ap = col(Y, ch, w)
                    # mix_out column (off critical path, on GpSimd)
                    nc.gpsimd.tensor_sub(col(MIX, ch, w), col(X, ch, w), t_ap)
                    # y = round(t) + (x - t)
                    nc.vector._custom_dve(
                        ops["quanty"], out=col(Y, ch, w), in0=t_ap,
                        in1=col(X, ch, w), s0=ROUND_C,
                    )

            def blk_out(dram, tile, sb):
                nc.sync.dma_start(
                    dram[sb * 128 : (sb + 1) * 128, :],
                    tile[:, sb * FB : (sb + 1) * FB],
                )

            # ---------------- schedule ----------------
            # Wave pipeline: wave w's scan overlaps the other wave's
            # transpose -> p_all -> q_fin tail on Tensor/Scalar.
            pp_cur = [None] * W
            for sb in range(NBLK):
                xmb_prep(sb)
                for w in range(W):
                    if sb == 0:
                        nc.scalar.copy(ycols(sb, w, 0, B), xmb_slice(sb, w))
                    else:
                        q_fin(sb, w, pp_cur[w])
                    if sb + 1 == NBLK - 1:
                        # dest block 5's contract-128 part only needs blocks
                        # 0..3: issue now so it runs under this wave's scan
                        pp = psum_e.tile([128, B * KW], fp32, tag="pp")
                        p_lo(sb + 1, w, pp)
                        pp_cur[w] = pp
                    steps(sb, w)
                    if sb + 1 < NBLK:
                        transpose_block(sb, w)
                        if sb + 1 == NBLK - 1:
                            p_hi(sb + 1, w, pp_cur[w])
                        else:
                            pp = psum_e.tile([128, B * KW], fp32, tag="pp")
                            p_lo(sb + 1, w, pp)
                            pp_cur[w] = pp
                blk_out(yt, Y, sb)
                blk_out(mot, MIX, sb)

    nc.compile()
    return nc


def get_nc(n_chunks=NPIX // 128):
    if n_chunks not in _CACHE:
        _CACHE[n_chunks] = _build(n_chunks)
    return _CACHE[n_chunks]


def make_core_inputs(x, mix, W, b):
    """Host-side layout prep. Returns list of per-core input dicts."""
    Wm = (W * np.tril(np.ones((C - 1, C), np.float32))).astype(np.float32)
    wt = np.zeros((C, C), np.float32)
    wt[:, 1:] = Wm.T  # wt[c, i] = Wm[i-1, c]
    # in-block triangle, negated, with +1 on the diagonal: the scan over
    # [y_0..y_{i-1}, q_i] then yields t_i = q_i - sum_j w_ij y_j directly
    wtri = np.zeros((NBLK, B, B), np.float32)
    for sb in range(NBLK):
        for i in range(1, B):
            ch = sb * B + i
            wtri[sb, i, :i] = -Wm[ch - 1, sb * B : sb * B + i]
            wtri[sb, i, i] = 1.0
    wtri = np.ascontiguousarray(
        np.broadcast_to(wtri.reshape(1, -1), (128, NBLK * B * B))
    )
    bpad = np.zeros((C,), np.float32)
    bpad[1:] = b
    ident = np.eye(128, dtype=np.float32)

    def to_slab(a):  # [C, H, W] -> [(b p), (k c)]
        return np.ascontiguousarray(
            a.reshape(NBLK, B, NPIX // 128, 128).transpose(0, 3, 2, 1)
        ).reshape(NBLK * 128, -1)

    in_maps = []
    for n in range(N):
        xtn = to_slab(x[n])
        mixn = to_slab(mix[n] + bpad[:, None, None])
        in_maps.append(
            {"xt": xtn, "mixt": mixn, "wt": wt, "wtri": wtri, "ident": ident}
        )
    return in_maps


def from_slab(a):  # [(b p), (k c)] -> [C, H, W]
    return (
        a.reshape(NBLK, 128, NPIX // 128, B)
        .transpose(0, 3, 2, 1)
        .reshape(C, H, Wd)
    )


def kernel(x, mix, W, b):
    from concourse.bass_utils import run_bass_kernel_spmd

    x = np.asarray(x, np.float32)
    mix = np.asarray(mix, np.float32)
    W = np.asarray(W, np.float32)
    b = np.asarray(b, np.float32)

    nc = get_nc()
    in_maps = make_core_inputs(x, mix, W, b)
    res = run_bass_kernel_spmd(nc, in_maps, list(range(N)))

    ybar = np.empty((N, C, H, Wd), np.float32)
    mix_out = np.empty((N, C, H, Wd), np.float32)
    for n in range(N):
        ybar[n] = from_slab(res.results[n]["yt"])
        mix_out[n] = from_slab(res.results[n]["mot"])
    mix_out[:, 0] = mix[:, 0]  # reference passes mix ch0 through exactly
    return ybar, mix_out

